# revision 64
# baseline (speedup 1.0000x reference)
"""Trainium2 Bass kernel for nn_ExLoss (tensor-parallel classifier loss).

Strategy (8 NeuronCores, SPMD):
  - V [100000, 256] is sharded along the class axis: 12500 classes/core.
    Each core computes its logits slice  inputs @ V_shard.T  -> [64, 12500]
    on the PE array (contraction dim on partitions; host passes transposed
    operands so all DMAs are contiguous), writes the slice to DRAM, and
    accumulates the per-row sum(exp(logit)) via ScalarE Exp+accumulate.
  - Matmul operands are fp16 (half the HBM traffic of fp32, full PE rate;
    logits accumulate in fp32 PSUM).  Decision margins of the hard-mining
    part are ~0.1, so fp16's ~3e-4 logits error cannot flip any of the
    discrete min/threshold decisions.
  - The B x B cosine-similarity hard-mining part is tiny and replicated on
    every core (x@x.T on PE, masked min/sum reductions on DVE with index
    masks precomputed on host from the int index/mask inputs).
  - Host combines the 8 partial softmax denominators (the "all-reduce" of
    the sharding hint), gathers target logits from the returned logits,
    and assembles the scalar loss exactly as the reference does.
"""

import numpy as np

B = 64
D = 256
C = 100000
P = 8
N = 32
NCORES = 8
CS = C // NCORES          # 12500 classes per core
CT = 500                  # classes per PSUM tile (<= 512 fp32 / bank)
NT = CS // CT             # 25 tiles per core
# DMA chunk schedule (columns per vT load): small first chunk so compute
# starts early, small last chunk so the final store tail is short.  Loads
# are issued in LOAD_ORDER: the final chunk right after the first, so its
# data is resident long before the tail.
CHUNKS = [1000, 2500, 2500, 2500, 2500, 1000, 500]
assert sum(CHUNKS) == CS and all(c % CT == 0 for c in CHUNKS)
LOAD_ORDER = [0, len(CHUNKS) - 1] + list(range(1, len(CHUNKS) - 1))
BIG = 1e30
THRESH = 0.3

MM_DTYPE = "float16"

_CACHE = {}


def _build_nc(matmul_dtype=MM_DTYPE, parts="all"):
    import concourse.tile as tile
    from concourse import bacc, mybir

    do_sim = parts in ("all", "sim")
    do_main = parts in ("all", "main")

    f32 = mybir.dt.float32
    mm_dt = getattr(mybir.dt, matmul_dtype)
    # 16-bit operands are converted on host (half the DMA bytes); fp32/
    # fp32r stream the fp32 bits and reinterpret on-chip
    narrow = mybir.dt.size(mm_dt) < 4
    in_dt = mm_dt if narrow else f32

    nc = bacc.Bacc(None, target_bir_lowering=False, debug=False)

    xT_d = nc.dram_tensor("xT", [D, B], in_dt, kind="ExternalInput")
    vT_d = nc.dram_tensor("vT", [D, CS], in_dt, kind="ExternalInput")
    posb_d = nc.dram_tensor("posbias", [B, B], f32, kind="ExternalInput")
    negc_d = nc.dram_tensor("negcnt", [B, B], f32, kind="ExternalInput")
    rinv_d = nc.dram_tensor("rinvT", [1, B], in_dt, kind="ExternalInput")
    logits_d = nc.dram_tensor("logits_out", [B, CS], f32,
                              kind="ExternalOutput")
    stats_d = nc.dram_tensor("stats_out", [B, 4], f32, kind="ExternalOutput")
    # per-tile exp sums; reduced on host (keeps the kernel tail short)
    sums_d = nc.dram_tensor("sums_out", [B, len(CHUNKS)], f32,
                            kind="ExternalOutput")

    with tile.TileContext(nc) as tc:
        with (
            tc.tile_pool(name="const", bufs=1) as cpool,
            tc.tile_pool(name="vin", bufs=7) as vpool,
            tc.tile_pool(name="lout", bufs=6) as lpool,
            tc.tile_pool(name="scr", bufs=2) as spool,
            tc.tile_pool(name="sim", bufs=1) as mpool,
            tc.tile_pool(name="psum", bufs=6, space="PSUM") as pspool,
            tc.tile_pool(name="psum_sim", bufs=1, space="PSUM") as psim,
        ):
            # ---- main-loop input prefetch goes FIRST on the SP queue so
            # the DMA pipeline starts immediately; small const loads go on
            # the Activation HWDGE queue instead ----
            vT_m = vT_d.ap().bitcast(mm_dt)
            # [256, CS] viewed as [128 partitions, 2 k-chunks, CS]: one
            # DMA loads both contraction halves of a chunk
            vT_k = vT_m.rearrange("(t p) c -> p t c", t=2)
            maxch = max(CHUNKS)
            # xt is tiny and every matmul needs it: first on the queue
            xt = cpool.tile([128, 2, B], mm_dt, name="xt")
            xT_m = xT_d.ap().bitcast(mm_dt)
            nc.sync.dma_start(xt[:, 0, :], xT_m[0:128, :])
            nc.sync.dma_start(xt[:, 1, :], xT_m[128:256, :])
            vts = {}
            if do_main:
                for ci in LOAD_ORDER:
                    csz = CHUNKS[ci]
                    c0 = sum(CHUNKS[:ci])
                    vt = vpool.tile([128, 2, maxch], mm_dt, name="vt")
                    nc.sync.dma_start(vt[:, :, 0:csz],
                                      vT_k[:, :, c0:c0 + csz])
                    vts[ci] = vt

            # ---- constants / replicated small inputs ----
            stats = cpool.tile([B, 4], f32, name="stats")
            nc.gpsimd.memset(stats[:], 0.0)

            if do_sim:
                posb = cpool.tile([B, B], f32, name="posb")
                nc.gpsimd.dma_start(posb[:], posb_d.ap()[:])
                negc = cpool.tile([B, B], f32, name="negc")
                nc.gpsimd.dma_start(negc[:], negc_d.ap()[:])

                # ---- B x B similarity / hard mining (replicated) ----
                # reciprocal row norms come in precomputed (rinvT input; 64
                # values of input prep) -- keeps Sqrt off the ScalarE so
                # the whole kernel uses a single activation table.
                # rinv feeds a matmul -> must sit at base partition 0:
                # allocate a full-partition tile and use row 0
                rinv128 = mpool.tile([128, B], mm_dt, name="rinv128")
                rinv = rinv128[0:1, :]
                nc.gpsimd.dma_start(rinv, rinv_d.ap().bitcast(mm_dt)[:])

                # outer(rinv, rinv) via K=1 matmul; gram = x @ x.T
                outerp = psim.tile([B, B], f32, name="outerp")
                nc.tensor.matmul(outerp[:], rinv, rinv, start=True, stop=True)
                outer = mpool.tile([B, B], f32, name="outer")
                nc.scalar.copy(outer[:], outerp[:])
                gram = psim.tile([B, B], f32, name="gram")
                nc.tensor.matmul(gram[:], xt[:, 0, :], xt[:, 0, :],
                                 start=True, stop=False)
                nc.tensor.matmul(gram[:], xt[:, 1, :], xt[:, 1, :],
                                 start=False, stop=True)
                sim = mpool.tile([B, B], f32, name="sim")
                nc.vector.tensor_mul(sim[:], gram[:], outer[:])

                # hardest positive: min over valid positive pairs
                # (tensor_tensor_reduce is a custom-DVE op that crashes
                # this runtime -- use standard two-op sequences instead)
                junk0 = mpool.tile([B, B], f32, name="junk0")
                hps = mpool.tile([B, 1], f32, name="hps")
                nc.vector.tensor_add(junk0[:], sim[:], posb[:])
                nc.vector.tensor_reduce(
                    hps[:], junk0[:], axis=mybir.AxisListType.X,
                    op=mybir.AluOpType.min,
                )
                # hard negatives: indicator sim > hps - THRESH
                thr = mpool.tile([B, 1], f32, name="thr")
                nc.vector.tensor_scalar_add(thr[:], hps[:], -THRESH)
                ind = mpool.tile([B, B], f32, name="ind")
                nc.vector.tensor_scalar(
                    ind[:], sim[:], thr[:], None, op0=mybir.AluOpType.is_gt
                )
                w = mpool.tile([B, B], f32, name="w")
                cnt = mpool.tile([B, 1], f32, name="cnt")
                nc.vector.tensor_mul(w[:], ind[:], negc[:])
                nc.vector.reduce_sum(cnt[:], w[:], axis=mybir.AxisListType.X)
                # softplus(sim) = ln(exp(sim) + 1); sim in [-1, 1] so this
                # is well-conditioned (no softplus table in this toolchain;
                # exp and ln share one activation table)
                esim = mpool.tile([B, B], f32, name="esim")
                nc.scalar.activation(
                    esim[:], sim[:], mybir.ActivationFunctionType.Exp
                )
                sp = mpool.tile([B, B], f32, name="sp")
                nc.scalar.activation(
                    sp[:], esim[:], mybir.ActivationFunctionType.Ln, bias=1.0
                )
                junk1 = mpool.tile([B, B], f32, name="junk1")
                hnsum = mpool.tile([B, 1], f32, name="hnsum")
                nc.vector.tensor_mul(junk1[:], w[:], sp[:])
                nc.vector.reduce_sum(
                    hnsum[:], junk1[:], axis=mybir.AxisListType.X
                )
                nc.vector.tensor_copy(stats[:, 1:2], hps[:])
                nc.vector.tensor_copy(stats[:, 2:3], hnsum[:])
                nc.vector.tensor_copy(stats[:, 3:4], cnt[:])

            if do_main:
                sums = cpool.tile([B, len(CHUNKS)], f32, name="sums")

                # ---- main loop: logits slices + exp-accumulate ----
                # exp+accумulate runs once per chunk on the SBUF staging
                # buffer (not per PSUM tile): 7 ScalarE ops instead of 25,
                # and PSUM banks are released right after the DVE copy
                c0 = 0
                for ci, csz in enumerate(CHUNKS):
                    vt = vts[ci]
                    last = ci == len(CHUNKS) - 1
                    lo = lpool.tile([B, maxch], f32, name="lo")
                    for j in range(csz // CT):
                        jsl = slice(j * CT, (j + 1) * CT)
                        ps = pspool.tile([B, CT], f32, name="ps")
                        nc.tensor.matmul(ps[:], xt[:, 0, :], vt[:, 0, jsl],
                                         start=True, stop=False)
                        nc.tensor.matmul(ps[:], xt[:, 1, :], vt[:, 1, jsl],
                                         start=False, stop=True)
                        nc.vector.tensor_copy(lo[:, jsl], ps[:])
                        if last:
                            # per-tile store keeps the final tail short
                            nc.sync.dma_start(
                                logits_d.ap()[:,
                                              c0 + j * CT:c0 + (j + 1) * CT],
                                lo[:, jsl],
                            )
                    scr = spool.tile([B, maxch], f32, name="scr")
                    nc.scalar.activation(
                        scr[:, 0:csz], lo[:, 0:csz],
                        mybir.ActivationFunctionType.Exp,
                        accum_out=sums[:, ci:ci + 1],
                    )
                    if not last:
                        nc.sync.dma_start(
                            logits_d.ap()[:, c0:c0 + csz], lo[:, 0:csz]
                        )
                    c0 += csz

                nc.sync.dma_start(sums_d.ap()[:], sums[:])

            nc.sync.dma_start(stats_d.ap()[:], stats[:])

    nc.compile()
    return nc


def get_nc(matmul_dtype=MM_DTYPE, parts="all"):
    key = ("nc", matmul_dtype, parts)
    if key not in _CACHE:
        _CACHE[key] = _build_nc(matmul_dtype, parts)
    return _CACHE[key]


def _prep_inputs(inputs, V, pos_idx, pos_mask, neg_idx, neg_mask,
                 matmul_dtype=MM_DTYPE):
    if matmul_dtype == "bfloat16":
        import ml_dtypes
        op_dt = np.dtype(ml_dtypes.bfloat16)
    elif matmul_dtype == "float16":
        op_dt = np.dtype(np.float16)
    else:
        op_dt = np.dtype(np.float32)
    x = np.asarray(inputs, dtype=np.float32)
    Vf = np.asarray(V, dtype=np.float32)
    pos_idx = np.asarray(pos_idx).astype(np.int64)
    pos_mask = np.asarray(pos_mask).astype(np.int64)
    neg_idx = np.asarray(neg_idx).astype(np.int64)
    neg_mask = np.asarray(neg_mask).astype(np.int64)

    xT = np.ascontiguousarray(x.T.astype(op_dt))

    posbias = np.full((B, B), BIG, np.float32)
    rows = np.repeat(np.arange(B), P)
    valid = pos_mask.ravel().astype(bool)
    posbias[rows[valid], pos_idx.ravel()[valid]] = 0.0

    negcnt = np.zeros((B, B), np.float32)
    np.add.at(
        negcnt,
        (np.repeat(np.arange(B), N), neg_idx.ravel()),
        neg_mask.ravel().astype(np.float32),
    )

    # reciprocal row norms (matches jnp: 1/max(||x_b||, 1e-12))
    nrm = np.maximum(
        np.sqrt(np.sum(x.astype(np.float64) ** 2, axis=1)), 1e-12
    ).astype(np.float32)
    rinvT = (1.0 / nrm).reshape(1, B).astype(op_dt)

    VT = Vf.T  # [D, C] view
    in_maps = []
    for k in range(NCORES):
        shard = np.ascontiguousarray(
            VT[:, k * CS:(k + 1) * CS].astype(op_dt)
        )
        in_maps.append({
            "xT": xT,
            "vT": shard,
            "posbias": posbias,
            "negcnt": negcnt,
            "rinvT": rinvT,
        })
    return in_maps


def kernel(inputs, V, targets, pos_idx, pos_mask, neg_idx, neg_mask):
    from concourse.bass_utils import run_bass_kernel_spmd

    nc = get_nc(MM_DTYPE)
    in_maps = _prep_inputs(inputs, V, pos_idx, pos_mask, neg_idx, neg_mask,
                           MM_DTYPE)
    res = run_bass_kernel_spmd(nc, in_maps, list(range(NCORES))).results

    logits = np.concatenate(
        [res[k]["logits_out"] for k in range(NCORES)], axis=1
    )

    # combine partial softmax denominators (host-side all-reduce of [B])
    s_tot = np.sum(
        np.stack([res[k]["sums_out"] for k in range(NCORES)], 0).astype(
            np.float64
        ),
        axis=(0, 2),
    )
    logZ = np.log(s_tot)

    tg = np.asarray(targets).astype(np.int64)
    t_logit = logits[np.arange(B), tg].astype(np.float64)
    bu_loss = np.mean(logZ - t_logit)

    st = res[0]["stats_out"].astype(np.float64)
    hpsims, hn_sum, cnt = st[:, 1], st[:, 2], st[:, 3]
    hp_loss = np.logaddexp(0.0, -hpsims)
    hn_loss = np.where(cnt > 0, hn_sum / np.maximum(cnt, 1.0), 0.0)
    h_loss = np.mean(hp_loss + hn_loss)

    loss = np.float32(bu_loss + h_loss)
    return loss, logits
